# revision 41
# baseline (speedup 1.0000x reference)
"""Multi-head self-attention (b=2, n=2048, emb=1024, heads=16) on 8 trn2 cores.

Sharding: core c = (b, hg) with b = c // 4, hg = c % 4. Data parallel over
batch, tensor parallel over head-groups (4 heads / 256 emb-cols per core).
Each core computes Q/K/V projections for its heads, full attention for its
heads, and a partial output projection ctx_hg @ Wo[:, hg_slice].T of shape
[2048, 1024]. The host sums the 4 partials per batch (Megatron row-parallel
reduce done on host) and adds the rank-1 bias term bv @ Wo.T + bo.

Device layout notes:
- Host pre-transposes x -> xT [emb, n] and weight slices so every matmul
  contracts over the partition dim.
- Q^T, K^T are produced in [dq, n] layout (dq = head-major), V in natural
  [n, dv] layout augmented with a ones column per head -> the ctx matmul
  ctxT[65, nq] = V_aug^T @ E^T produces softmax row-sums in row 64 for free.
- exp(S^T) runs on ACT straight out of PSUM in up-to-1536-wide instructions;
  softmax normalization is deferred to the small ctx^T tile.
- All matmuls run in float16 (1 cyc/col on PE + fast weight load; 10-bit
  mantissa keeps the overall error ~7e-4 scale-relative, validated vs fp32).
- q/k biases are added on-device (free, fused into the PSUM->SBUF copy);
  v/o biases are exactly the rank-1 host-side term above.
"""

import os
import sys

for _p in ("/opt/trn_rl_repo", "/root/.axon_site/_ro/trn_rl_repo"):
    if os.path.isdir(_p) and _p not in sys.path:
        sys.path.append(_p)

import numpy as np

import concourse.bass as bass  # noqa: F401  (engine types pulled via nc)
import concourse.mybir as mybir
import concourse.tile as tile
from concourse import bacc
from concourse.bass_utils import run_bass_kernel_spmd

B, N, EMB, HEADS, HD = 2, 2048, 1024, 16, 64
N_CORES = 8
TP = 4                      # head-group shards per batch
DQ = EMB // TP              # 256 emb-cols (4 heads) per core
SCALE = HD ** -0.5          # 0.125

F32 = mybir.dt.float32
F16 = mybir.dt.float16
FP = mybir.ActivationFunctionType

NQ = 512                    # nq chunk for projections / out-proj (moving free dim)
NJ = N // NQ                # 4 nq chunks
NQA = 256                   # nq chunk for attention (so 6 nk-chunks fit one exp)
NJA = N // NQA              # 8 attention nq chunks
NKC = 128                   # nk chunk (ctx contraction)
NT = N // NKC               # 16 nk chunks
KC = EMB // 128             # 8 e chunks
# nk-chunk groups per exp instruction (4 x 256 -> 1024-wide exps).
# PSUM budget (8 banks): pp 2 + s0 2 + s1 2 + c0 1 + c1 1. pp is
# double-buffered so projection psum groups never head-of-line-block the
# in-order PE queue while attention S matmuls are behind them.
T_GROUPS_H = (
    [tuple(range(0, 4)), tuple(range(4, 8)), tuple(range(8, 12)),
     tuple(range(12, 16))],
    [tuple(range(0, 4)), tuple(range(4, 8)), tuple(range(8, 12)),
     tuple(range(12, 16))],
)


def build_program():
    """Build + compile the single SPMD program all 8 cores run."""
    nc = bacc.Bacc("TRN2", target_bir_lowering=False, debug=False,
                   num_devices=N_CORES)

    xT = nc.dram_tensor("xT", [EMB, N], F16, kind="ExternalInput").ap()
    wqT = nc.dram_tensor("wqT", [EMB, DQ], F16, kind="ExternalInput").ap()
    wkT = nc.dram_tensor("wkT", [EMB, DQ], F16, kind="ExternalInput").ap()
    wvT = nc.dram_tensor("wvT", [EMB, DQ], F16, kind="ExternalInput").ap()
    woT = nc.dram_tensor("woT", [DQ, EMB], F16, kind="ExternalInput").ap()
    bqd = nc.dram_tensor("bq_s", [DQ], F32, kind="ExternalInput").ap()
    bkd = nc.dram_tensor("bk_s", [DQ], F32, kind="ExternalInput").ap()
    out_part = nc.dram_tensor("out_part", [N, EMB], F32,
                              kind="ExternalOutput").ap()

    with tile.TileContext(nc) as tc:
        with (
            tc.tile_pool(name="const", bufs=1) as const,
            tc.tile_pool(name="xp", bufs=24) as xp,
            tc.tile_pool(name="persist", bufs=1) as persist,
            tc.tile_pool(name="epool", bufs=2) as epool,
            tc.tile_pool(name="npool", bufs=2) as npool,
            tc.tile_pool(name="opool", bufs=4) as opool,
            # PSUM static budget (8 banks): pp 1 + s0 3 + s1 3 + c 1
            tc.tile_pool(name="ppool", bufs=2, space="PSUM") as ppool,
            tc.tile_pool(name="spool", bufs=1, space="PSUM") as spool,
            tc.tile_pool(name="cpool", bufs=1, space="PSUM") as cpool,
        ):
            # ---- constants ----
            # per-k-chunk weight DMAs: the first projection matmul only
            # depends on its own 64KB slice, not the whole weight
            wq_sb = const.tile([128, KC, DQ], F16, tag="wq")
            wk_sb = const.tile([128, KC, DQ], F16, tag="wk")
            wv_sb = const.tile([128, KC, DQ], F16, tag="wv")
            for k in range(KC):
                nc.sync.dma_start(out=wk_sb[:, k, :], in_=wkT.rearrange(
                    "(k p) d -> k p d", p=128)[k])
                nc.sync.dma_start(out=wv_sb[:, k, :], in_=wvT.rearrange(
                    "(k p) d -> k p d", p=128)[k])
                nc.sync.dma_start(out=wq_sb[:, k, :], in_=wqT.rearrange(
                    "(k p) d -> k p d", p=128)[k])
            # wo is needed only by the out-projection (~60us in) — its DMA
            # is deferred into the filler stream to keep startup queues clear
            wo_sb = const.tile([128, 2, EMB], F16, tag="wo")
            bq_sb = const.tile([128, 2], F32, tag="bq")
            nc.sync.dma_start(out=bq_sb, in_=bqd.rearrange("(m p) -> p m", p=128))
            bk_sb = const.tile([128, 2], F32, tag="bk")
            nc.sync.dma_start(out=bk_sb, in_=bkd.rearrange("(m p) -> p m", p=128))

            # ---- persistent activations ----
            qT = [persist.tile([128, N], F16, tag=f"qT{p}", name=f"qT{p}") for p in range(2)]
            kT = [persist.tile([128, N], F16, tag=f"kT{p}", name=f"kT{p}") for p in range(2)]
            ctxT = [persist.tile([128, N], F16, tag=f"ctxT{p}", name=f"ctxT{p}") for p in range(2)]
            # V for all 4 local heads: [nk-part, t, head*65 + (0:64 | ones)]
            v_all = persist.tile([128, NT, 4 * (HD + 1)], F16, tag="v")
            for h in range(4):
                nc.vector.memset(v_all[:, :, h * 65 + 64], 1.0)

            add, mult = mybir.AluOpType.add, mybir.AluOpType.mult

            # ---- projection building blocks ----
            # Each returns/consumes one PSUM accumulation group, small enough
            # to slot between attention groups without starving ACT.
            _xts = {}

            def load_x_chunk(pn):
                p, n = pn
                xts = []
                for k in range(KC):
                    xt = xp.tile([128, NQ], F16, tag="xt", name="xt")
                    nc.sync.dma_start(
                        out=xt,
                        in_=xT[k * 128:(k + 1) * 128, n * NQ:(n + 1) * NQ])
                    xts.append(xt)
                _xts[pn] = xts
                return xts

            def kq_group(p, n, wsb, bsb, dst):
                xts = _xts[(p, n)]
                ps = ppool.tile([128, NQ], F32, tag="pp", name="kqp")
                for k in range(KC):
                    nc.tensor.matmul(
                        ps, wsb[:, k, p * 128:(p + 1) * 128],
                        xts[k], start=(k == 0), stop=(k == KC - 1))
                nc.vector.tensor_tensor(
                    out=dst[p][:, n * NQ:(n + 1) * NQ], in0=ps,
                    in1=bsb[:, p:p + 1].broadcast_to([128, NQ]), op=add)

            def v_group(p, n, tl):
                xts = _xts[(p, n)]
                t = n * 4 + tl
                ps = ppool.tile([128, NQ], F32, tag="pp", name="vp")
                for k in range(KC):
                    nc.tensor.matmul(
                        ps[:, 0:128], xts[k][:, tl * 128:(tl + 1) * 128],
                        wv_sb[:, k, p * 128:(p + 1) * 128],
                        start=(k == 0), stop=(k == KC - 1))
                vv = v_all[:, t, :].rearrange("p (h c) -> p h c", c=65)
                nc.vector.tensor_copy(
                    out=vv[:, 2 * p:2 * p + 2, 0:64],
                    in_=ps[:, 0:128].rearrange("p (h c) -> p h c", c=64))

            def proj_fillers(p):
                # per n-chunk: K + 4 V + Q as 6 filler parcels; the x-chunk
                # DMAs are issued one n-chunk ahead so PE never head-of-line
                # blocks on a fresh load
                out = [lambda p=p: load_x_chunk((p, 0)),
                       lambda p=p: load_x_chunk((p, 1))]
                for n in range(NJ):
                    out.append(lambda p=p, n=n: kq_group(p, n, wk_sb, bk_sb, kT))
                    for tl in range(4):
                        out.append(lambda p=p, n=n, tl=tl: v_group(p, n, tl))
                    out.append(lambda p=p, n=n: (
                        kq_group(p, n, wq_sb, bq_sb, qT),
                        _xts.pop((p, n))))
                    if n + 2 < NJ:
                        out.insert(-4, lambda p=p, n=n: load_x_chunk((p, n + 2)))
                return out

            # pair-0 projections run up front (serial ACT-idle prefix
            # ~20us; attention needs all of kT0/v before it can start)
            for f in proj_fillers(0):
                f()

            # ---- attention (per head-pair p, nq chunk j of 256) ----
            # Software-pipelined: ctx matmuls for group g are emitted after
            # the S/exp of group g+1, so PE always has ready work while ACT
            # streams wide exps; heads alternate as the natural PSUM
            # ping-pong for the S tiles. The ctx PSUM bank is released by one
            # quick copy to SBUF; the reciprocal-normalize then runs fully
            # off the critical path on DVE/GpSimd.
            o_tiles = {}

            def out_proj_parcel(m, eo):
                # one (m, eo) parcel of the output projection: both pairs
                # accumulated in PSUM (pp tag), one copy, DMA after eo=1.
                # Runs as filler once ctxT1's m-window is final.
                if eo == 0:
                    o_tiles[m] = opool.tile([128, EMB], F32, tag="o", name="o")
                o = o_tiles[m]
                po = ppool.tile([128, NQ], F32, tag="pp", name="po")
                for kp in range(2):
                    nc.tensor.matmul(
                        po, ctxT[kp][:, m * 128:(m + 1) * 128],
                        wo_sb[:, kp, eo * NQ:(eo + 1) * NQ],
                        start=(kp == 0), stop=(kp == 1))
                nc.vector.tensor_copy(o[:, eo * NQ:(eo + 1) * NQ], po)
                if eo == 1:
                    nc.sync.dma_start(
                        out=out_part[m * 128:(m + 1) * 128, :], in_=o)

            from collections import deque
            fillers = deque()

            for p in range(2):
                if p == 0:
                    # deferred wo load + pair-1 projections trickle through
                    # pair-0's attention window
                    fillers.append(lambda: nc.sync.dma_start(
                        out=wo_sb,
                        in_=woT.rearrange("(k p) e -> p k e", p=128)))
                    fillers.extend(proj_fillers(1))

                # pace: spread this window's fillers over its 8 j-iterations,
                # popping evenly between attention work items (8 per j)
                for j in range(NJA):
                    cps = [cpool.tile([HD + 1, NQA], F32, tag=f"c{h}",
                                      name=f"c{h}") for h in range(2)]

                    def s_mms(g, h):
                        lo = 64 * h
                        sp = spool.tile([128, len(g), NQA], F32,
                                        tag=f"s{h}", name=f"s{h}")
                        for i, t in enumerate(g):
                            nc.tensor.matmul(
                                sp[:, i, :],
                                kT[p][lo:lo + 64, t * 128:(t + 1) * 128],
                                qT[p][lo:lo + 64, j * NQA:(j + 1) * NQA],
                                start=True, stop=True)
                        return sp

                    def exp_act(sp, g, h):
                        e = epool.tile([128, len(g), NQA], F16,
                                       tag=f"e{h}", name=f"e{h}")
                        nc.scalar.activation(e, sp, FP.Exp, scale=SCALE)
                        return e

                    def ctx_mms(e, g, h):
                        hloc = 2 * p + h
                        for i, t in enumerate(g):
                            nc.tensor.matmul(
                                cps[h],
                                v_all[:, t, hloc * 65:(hloc + 1) * 65],
                                e[:, i, :],
                                start=(t == 0), stop=(t == NT - 1))

                    # interleave the two heads' group streams; ctx trails by
                    # one work item so PE always has ready matmuls queued.
                    # Filler parcels (projections / out-proj) are popped
                    # between work items, paced to spread over remaining j's.
                    work = []
                    for gi in range(max(len(T_GROUPS_H[0]), len(T_GROUPS_H[1]))):
                        for h in range(2):
                            if gi < len(T_GROUPS_H[h]):
                                work.append((T_GROUPS_H[h][gi], h))
                    n_pop = -(-len(fillers) // (NJA - j))  # ceil
                    prev = None
                    for wi, (g, h) in enumerate(work):
                        sp = s_mms(g, h)
                        cur = (exp_act(sp, g, h), g, h)
                        if prev is not None:
                            ctx_mms(*prev)
                        prev = cur
                        if fillers and wi < n_pop:
                            fillers.popleft()()
                    ctx_mms(*prev)
                    for _ in range(8, n_pop):
                        if fillers:
                            fillers.popleft()()

                    # normalize: ctx^T[0:64] * (1 / rowsum); rowsum in row 64.
                    # First copy out of PSUM (frees the ctx bank), then the
                    # slow reciprocal chain runs out of SBUF asynchronously.
                    for h in range(2):
                        cs = npool.tile([HD + 1, NQA], F32, tag="cs", name="cs")
                        nc.vector.tensor_copy(cs, cps[h])
                        # partition_broadcast reads physical partition 0, so
                        # stage the rowsum row there first
                        rs = npool.tile([1, NQA], F32, tag="rs", name="rs")
                        nc.vector.tensor_copy(rs, cs[64:65, :])
                        rb = npool.tile([64, NQA], F32, tag="rb", name="rb")
                        nc.gpsimd.partition_broadcast(rb, rs)
                        rc = npool.tile([64, NQA], F32, tag="rc", name="rc")
                        nc.vector.reciprocal(rc, rb)
                        nc.vector.tensor_tensor(
                            out=ctxT[p][h * 64:(h + 1) * 64,
                                        j * NQA:(j + 1) * NQA],
                            in0=cs[0:64, :], in1=rc, op=mult)
                    if p == 1:
                        # ctxT1 columns for this j are final -> out-proj
                        # parcels for the covered m-chunks can run
                        for m in (2 * j, 2 * j + 1):
                            for eo in range(2):
                                fillers.append(
                                    lambda m=m, eo=eo:
                                    out_proj_parcel(m, eo))
            while fillers:
                fillers.popleft()()

    nc.compile()
    return nc


_NC_CACHE = {}


def _get_program():
    if "nc" not in _NC_CACHE:
        _NC_CACHE["nc"] = build_program()
    return _NC_CACHE["nc"]


def make_in_maps(x, Wq, bq, Wk, bk, Wv, bv, Wo, bo):
    x = np.asarray(x)
    xTs = [np.ascontiguousarray(x[b].T.astype(np.float16)) for b in range(B)]
    in_maps = []
    for c in range(N_CORES):
        b, hg = divmod(c, TP)
        sl = slice(hg * DQ, (hg + 1) * DQ)
        in_maps.append({
            "xT": xTs[b],
            "wqT": np.ascontiguousarray(np.asarray(Wq, np.float16)[sl, :].T),
            "wkT": np.ascontiguousarray(np.asarray(Wk, np.float16)[sl, :].T),
            "wvT": np.ascontiguousarray(np.asarray(Wv, np.float16)[sl, :].T),
            "woT": np.ascontiguousarray(np.asarray(Wo, np.float16)[:, sl].T),
            "bq_s": np.ascontiguousarray(np.asarray(bq, np.float32)[sl]),
            "bk_s": np.ascontiguousarray(np.asarray(bk, np.float32)[sl]),
        })
    return in_maps


def assemble_output(results, Wv_bias_term):
    out = np.empty((B, N, EMB), np.float32)
    for b in range(B):
        acc = results[b * TP]["out_part"].astype(np.float32)
        for g in range(1, TP):
            acc = acc + results[b * TP + g]["out_part"]
        out[b] = acc + Wv_bias_term
    return out


def kernel(x, Wq, bq, Wk, bk, Wv, bv, Wo, bo):
    nc = _get_program()
    in_maps = make_in_maps(x, Wq, bq, Wk, bk, Wv, bv, Wo, bo)
    res = run_bass_kernel_spmd(nc, in_maps, list(range(N_CORES)))
    bias_term = (np.asarray(bv, np.float32) @ np.asarray(Wo, np.float32).T
                 + np.asarray(bo, np.float32))
    return assemble_output(res.results, bias_term)
